# revision 1
# baseline (speedup 1.0000x reference)
"""Trainium2 Bass kernel for DiffSortNet (differentiable bitonic sort network).

Full inputs in, full outputs out. Pure data parallel over 8 NeuronCores
(batch 512 -> 64 per core). The one-hot selector matrices are compile-time
constants of the bitonic network for n=256, so the kernel derives the
(lo, hi, direction) structure itself; only `vectors` goes to the device.

Math (per batch b, layer with pair distance m):
    pairs (lo, hi=lo+m), direction flag = bit_{block+1}(lo)
    dv = (v[hi]-v[lo]) * (flag ? -1 : +1);  q = arctan(10*dv)/pi + 0.5
    X[:,lo], X[:,hi] = H + q*(L-H), L - q*(L-H)      (L/H = old X cols)

Performance structure:
  * window sparsity: after block bi, column j of X is supported only on
    rows i inside the aligned 2^(bi+1)-window of j, so each butterfly op
    only touches i inside the (window of the pair) -> ~2.4x fewer elements.
  * 4-pass in-place update via swapped writes: newLo is written AT the old
    hi position and newHi at the old lo position, so each op reads/writes
    the same columns (no WAR hazard, no 5th pass). This leaves the columns
    physically permuted by XOR mask M (M ^= m per layer); bookkeeping is
    compile-time. Four layers (one for each m with odd multiplicity:
    m=2,8,32,128) instead use the 5-pass non-swapping form so the final
    M is 0 and the output DMA is straight.
  * v_perm (the vector entries in current physical column order) is
    maintained by two small copies per swapping layer, so dv/q for layer t
    can be computed with plain strided slices.

SBUF layout: partition p = h*64 + b (h = i-half of X), free = (i_lo, j).
q broadcasts along i via a zero-stride AP dim.
"""
import math
import sys
from contextlib import ExitStack

sys.path.insert(0, "/opt/trn_rl_repo")

import numpy as np

import concourse.bacc as bacc
import concourse.bass as bass
import concourse.mybir as mybir
import concourse.tile as tile
from concourse.bass_utils import run_bass_kernel_spmd

N = 256
B_FULL = 512
N_CORES = 8
B_LOC = B_FULL // N_CORES  # 64
STEEP = 10.0
FP = mybir.dt.float32
LOG2N = 8
XFREE = 128 * N  # x tile free size (per-partition f32 elements)
FD_CAP = 2048    # max free elements per butterfly instruction (scratch cap)


def _layer_structure(n=N):
    """[(block, layer, m, flag_bit, swap)] for the 36 layers. `swap` marks
    layers using the 4-pass swapped-write form; one layer per odd-multiplicity
    m (2, 8, 32, 128) uses the 5-pass form so XOR masks cancel to 0."""
    out = []
    noswap = {(1, 0), (3, 0), (5, 0), (7, 0)}
    for bi in range(int(math.log2(n))):
        for li in range(bi + 1):
            m = 2 ** (bi - li)
            out.append((bi, li, m, bi + 1, (bi, li) not in noswap))
    return out


LAYERS = _layer_structure()
L = len(LAYERS)  # 36


def emit(tc, v_in, x_out, n_layers=L):
    nc = tc.nc
    O = mybir.AluOpType
    A = mybir.ActivationFunctionType
    with ExitStack() as ctx:
        pool = ctx.enter_context(tc.tile_pool(name="main", bufs=1))
        scratch = ctx.enter_context(tc.tile_pool(name="scr", bufs=1))
        bfly = ctx.enter_context(tc.tile_pool(name="bfly", bufs=2))

        # ---- load vectors, replicated across h (p = h*64 + b) ----
        va = pool.tile([128, N], FP, tag="va")
        vb = pool.tile([128, N], FP, tag="vb")
        nc.sync.dma_start(va[0:B_LOC, :], v_in)
        nc.sync.dma_start(va[B_LOC:128, :], v_in)

        # ---- q_all[p, t*128+k]: dv with sign fold + range-reduced arctan --
        # v_perm state: v_cur[phys col c] = v[c ^ M]; M advances on swap layers
        q_all = pool.tile([128, n_layers * 128], FP, tag="qa")
        v_cur, v_nxt = va, vb
        M = 0
        Q_CHUNK = 6
        for t0 in range(0, n_layers, Q_CHUNK):
            tn = min(Q_CHUNK, n_layers - t0)
            cw = tn * 128
            z = scratch.tile([128, Q_CHUNK * 128], FP, tag="z")
            t1 = scratch.tile([128, Q_CHUNK * 128], FP, tag="t1")
            t2 = scratch.tile([128, Q_CHUNK * 128], FP, tag="t2")
            t3 = scratch.tile([128, Q_CHUNK * 128], FP, tag="t3")
            mk = scratch.tile([128, Q_CHUNK * 128], mybir.dt.uint8, tag="mk")
            for ti in range(tn):
                t = t0 + ti
                bi, li, m, fb, swap = LAYERS[t]
                ngrp = N // (2 * m)
                lm = bi - li  # log2(m)
                Mlm = (M >> lm) & 1
                vv = v_cur[:].rearrange("p (g r j) -> p g r j", g=ngrp, r=2)
                vlo = vv[:, :, Mlm, :]      # logical-lo values  [p, g, m]
                vhi = vv[:, :, 1 - Mlm, :]  # logical-hi values
                dv_t = z[:, ti * 128 : (ti + 1) * 128].rearrange(
                    "p (g j) -> p g j", g=ngrp
                )
                if fb < LOG2N:
                    # flag = bit_fb(logical lo) = bit gbit of physical g,
                    # XOR M_fb; gbit = fb - lm - 1
                    gbit = fb - lm - 1
                    go = 2 ** gbit
                    Mfb = (M >> fb) & 1
                    ga = dv_t.rearrange("p (a f g) j -> p a f g j", f=2, g=go)
                    vl = vlo.rearrange("p (a f g) j -> p a f g j", f=2, g=go)
                    vh = vhi.rearrange("p (a f g) j -> p a f g j", f=2, g=go)
                    asc, dsc = Mfb, 1 - Mfb
                    nc.vector.tensor_tensor(
                        ga[:, :, asc], vh[:, :, asc], vl[:, :, asc], O.subtract
                    )
                    nc.vector.tensor_tensor(
                        ga[:, :, dsc], vl[:, :, dsc], vh[:, :, dsc], O.subtract
                    )
                else:
                    nc.vector.tensor_tensor(dv_t, vhi, vlo, O.subtract)
                if swap:
                    # v_nxt[c] = v_cur[c ^ m]
                    vn = v_nxt[:].rearrange("p (g r j) -> p g r j", g=ngrp, r=2)
                    nc.scalar.copy(vn[:, :, 0, :], vv[:, :, 1, :])
                    nc.scalar.copy(vn[:, :, 1, :], vv[:, :, 0, :])
                    v_cur, v_nxt = v_nxt, v_cur
                    M ^= m

            # q = arctan(10*z)/pi + 0.5, range-reduced:
            #   |10z| < 1 : arctan(10z);  else sign(z)*(pi/2 - arctan(1/|10z|))
            # 1/w via exp(-ln(max(w,1))) (ACT Reciprocal banned, DVE recip slow)
            qc = q_all[:, t0 * 128 : t0 * 128 + cw]
            zc, t1c, t2c, t3c = z[:, :cw], t1[:, :cw], t2[:, :cw], t3[:, :cw]
            nc.vector.tensor_scalar(t1c, zc, -0.1, 0.1, O.max, O.min)
            nc.scalar.activation(qc, t1c, A.Arctan, bias=0.0, scale=STEEP)
            nc.scalar.activation(t1c, zc, A.Abs, bias=0.0, scale=STEEP)
            nc.gpsimd.tensor_scalar(t2c, t1c, 1.0, None, O.max)
            nc.scalar.activation(t2c, t2c, A.Ln)
            nc.scalar.activation(t2c, t2c, A.Exp, bias=0.0, scale=-1.0)
            nc.scalar.activation(t2c, t2c, A.Arctan)
            nc.scalar.activation(t3c, zc, A.Sign)
            nc.gpsimd.tensor_tensor(t2c, t3c, t2c, O.mult)
            nc.vector.scalar_tensor_tensor(
                t2c, t3c, float(math.pi / 2), t2c, O.mult, O.subtract
            )
            nc.gpsimd.tensor_scalar(mk[:, :cw], t1c, 1.0, None, O.is_lt)
            nc.vector.copy_predicated(t2c, mk[:, :cw], qc)
            nc.scalar.activation(qc, t2c, A.Copy,
                                 bias=0.5, scale=float(1.0 / math.pi))

        # ---- init X = identity ----
        x = pool.tile([128, XFREE], FP, tag="x")
        nc.vector.memset(x[:], 0.0)
        x3 = x[:].rearrange("p (i j) -> p i j", j=N)
        xh = x3.rearrange("(h b) i j -> h b i j", h=2)
        diag0 = xh[0].rearrange("b i j -> b (i j)")[:, :: N + 1]
        diag1 = xh[1].rearrange("b i j -> b (i j)")[:, 128 :: N + 1][:, :128]
        nc.vector.memset(diag0, 1.0)
        nc.vector.memset(diag1, 1.0)

        xt = x[:].tensor  # handle for raw APs
        NL = n_layers * 128
        qt = q_all[:].tensor

        def _mk(tensor, off, pdim, dims):
            dims = [d for d in dims if d[1] != 1]  # HW ISA: <=3 free dims
            assert len(dims) <= 3, dims
            return bass.AP(tensor, off, [pdim] + dims)

        def x_ap(h, w, role, m, d0, ce, W0, nw_op):
            """X operand AP: windows [W0, W0+nw_op) (diagonal i/j step),
            delta chunk [d0, d0+ce), role = bit_lm column of each pair."""
            if w <= 128:
                # W0 is h-relative: i base = W0*w, col base = W0*w + 128h
                off = ((64 * h) * XFREE + (W0 * w) * N + W0 * w + 128 * h
                       + d0 * N + role * m)
                dims = [[w * N + w, nw_op], [N, ce],
                        [2 * m, w // (2 * m)], [1, m]]
                return _mk(xt, off, [XFREE, 64], dims)
            off = d0 * N + role * m
            dims = [[N, ce], [2 * m, N // (2 * m)], [1, m]]
            return _mk(xt, off, [XFREE, 128], dims)

        def s_ap(tile_h, h, w, m, ce, nw_op):
            """Scratch AP (packed), partition-sliced to match x_ap."""
            st = tile_h[:].tensor
            if w <= 128:
                dims = [[ce * (w // 2), nw_op], [w // 2, ce],
                        [m, w // (2 * m)], [1, m]]
                return _mk(st, (64 * h) * FD_CAP, [FD_CAP, 64], dims)
            dims = [[128, ce], [m, N // (2 * m)], [1, m]]
            return _mk(st, 0, [FD_CAP, 128], dims)

        def q_ap(h, w, m, ce, t, W0, nw_op):
            if w <= 128:
                # h=1 covers the second half of the pair index range
                off = (64 * h) * NL + t * 128 + h * 64 + W0 * (w // 2)
                dims = [[w // 2, nw_op], [0, ce], [m, w // (2 * m)], [1, m]]
                return _mk(qt, off, [NL, 64], dims)
            dims = [[0, ce], [m, N // (2 * m)], [1, m]]
            return _mk(qt, t * 128, [NL, 128], dims)

        # ---- butterfly layers ----
        M = 0
        for t in range(n_layers):
            bi, li, m, fb, swap = LAYERS[t]
            w = 2 ** (bi + 1)
            lm = bi - li
            Mlm = (M >> lm) & 1
            if w <= 128:
                hs = (0, 1)
                nw = 128 // w
                dext = w            # delta extent per window
            else:
                hs = (None,)
                nw = 1
                dext = 128
            # HW ISA allows 3 free dims; loop windows if all 4 nontrivial
            nontriv = sum(1 for c in (nw, dext, w // (2 * m), m) if c > 1)
            nw_op = 1 if (nontriv > 3) else nw
            n_wops = nw // nw_op
            perchunk = nw_op * (w // 2)  # FD per delta row
            ce_max = max(1, FD_CAP // perchunk)
            for h in hs:
                hh = h if h is not None else 0
                for wi in range(n_wops):
                    W0 = wi * nw_op
                    d0 = 0
                    while d0 < dext:
                        ce = min(ce_max, dext - d0)
                        sL = x_ap(hh, w, Mlm, m, d0, ce, W0, nw_op)
                        sH = x_ap(hh, w, 1 - Mlm, m, d0, ce, W0, nw_op)
                        qb = q_ap(hh, w, m, ce, t, W0, nw_op)
                        d = bfly.tile([128, FD_CAP], FP, tag="d")
                        qd = bfly.tile([128, FD_CAP], FP, tag="qd")
                        dp = s_ap(d, hh, w, m, ce, nw_op)
                        qdp = s_ap(qd, hh, w, m, ce, nw_op)
                        nc.vector.tensor_tensor(dp, sL, sH, O.subtract)
                        nc.vector.tensor_tensor(qdp, dp, qb, O.mult)
                        if swap:
                            # newLo at H slot, newHi at L slot (no WAR hazard)
                            nc.vector.tensor_tensor(sH, sH, qdp, O.add)
                            nc.vector.tensor_tensor(sL, sL, qdp, O.subtract)
                        else:
                            d2 = scratch.tile([128, FD_CAP], FP, tag="d2")
                            d2p = s_ap(d2, hh, w, m, ce, nw_op)
                            nc.vector.tensor_tensor(d2p, dp, qdp, O.subtract)
                            nc.vector.tensor_tensor(sL, sH, qdp, O.add)
                            nc.vector.tensor_tensor(sH, sH, d2p, O.add)
                        d0 += ce
            if swap:
                M ^= m
        assert n_layers < L or M == 0, f"final XOR mask {M} != 0"

        # ---- write out (one DMA per h half) ----
        oh = x_out.rearrange("b (h i) j -> h b (i j)", h=2)
        nc.sync.dma_start(oh[0], x[0:B_LOC, :])
        nc.sync.dma_start(oh[1], x[B_LOC:128, :])


def build_nc(n_layers=L):
    nc = bacc.Bacc("TRN2", target_bir_lowering=False, debug=False)
    v_in = nc.declare_dram_parameter("vectors", [B_LOC, N], FP, isOutput=False)
    x_out = nc.declare_dram_parameter("out", [B_LOC, N, N], FP, isOutput=True)
    with tile.TileContext(nc) as tc:
        emit(tc, v_in[:], x_out[:], n_layers=n_layers)
    nc.finalize()
    return nc


_NC_CACHE = {}


def kernel(**inputs) -> np.ndarray:
    vectors = np.asarray(inputs["vectors"], dtype=np.float32)
    assert vectors.shape == (B_FULL, N)
    if "default" not in _NC_CACHE:
        _NC_CACHE["default"] = build_nc()
    nc = _NC_CACHE["default"]
    in_maps = [
        {"vectors": vectors[c * B_LOC : (c + 1) * B_LOC]} for c in range(N_CORES)
    ]
    res = run_bass_kernel_spmd(nc, in_maps, core_ids=list(range(N_CORES)))
    out = np.concatenate([res.results[c]["out"] for c in range(N_CORES)], axis=0)
    return out


if __name__ == "__main__":
    rng = np.random.default_rng(0)
    v = rng.normal(size=(B_FULL, N)).astype(np.float32)
    o = kernel(vectors=v)
    print("kernel output shape:", o.shape, o.dtype)



# revision 3
# speedup vs baseline: 2.2352x; 2.2352x over previous
"""Trainium2 Bass kernel for DiffSortNet (differentiable bitonic sort network).

Full inputs in, full outputs out. Pure data parallel over 8 NeuronCores
(batch 512 -> 64 per core). Selector matrices are compile-time constants of
the bitonic network for n=256; the kernel derives (lo, hi, direction) itself.

Math (per batch b, layer with pair distance m):
    pairs (lo, hi=lo+m), direction flag = bit_{block+1}(lo)
    dv = (v[hi]-v[lo]) * (flag ? -1 : +1);  q = arctan(10*dv)/pi + 0.5
    X[:,lo], X[:,hi] = H + q*(L-H), L - q*(L-H)      (L/H = old X cols)

Performance structure (v2 vs the old h-split kernel):
  * layout: partition p = jh*64 + b  (jh = column half of X, b = batch).
    Free dim = (ir, jl): jl = j & 127, and ir = i XOR (jh*128) -- the i halves
    are swapped on the jh=1 partitions so every window's free-dim offsets are
    partition-uniform. All 35 within-half layers then run on all 128
    partitions in a single instruction stream (the old kernel issued each op
    twice on 64-partition halves).
  * fp16 X and q: DVE's 2x_1p fast mode (2-byte dtype + packed innermost AP)
    doubles tensor_tensor throughput for every m>=2 butterfly op.
  * window sparsity: after block bi, column j of X is supported only on
    rows i inside the aligned 2^(bi+1)-window of j.
  * swapped in-place writes (newLo at hi slot) avoid a 5th pass; 4 layers
    use the 5-pass non-swap form so the XOR bookkeeping cancels (M_final=0).
  * layer (7,0) (pairs span column halves) exploits disjoint supports
    (L lives on i<128, H on i>=128) to run as 4 mults/subs + 2 copies.
"""
import math
import sys
from contextlib import ExitStack

sys.path.insert(0, "/opt/trn_rl_repo")

import numpy as np

import concourse.bacc as bacc
import concourse.bass as bass
import concourse.mybir as mybir
import concourse.tile as tile
from concourse.bass_utils import run_bass_kernel_spmd

N = 256
B_FULL = 512
N_CORES = 8
B_LOC = B_FULL // N_CORES  # 64
STEEP = 10.0
FP = mybir.dt.float32
HP = mybir.dt.float16
LOG2N = 8
XF = 256 * 128  # x free size per partition: (ir, jl)
FD_CAP = 8192   # max free elements per butterfly instruction


def _layer_structure(n=N):
    out = []
    noswap = {(1, 0), (3, 0), (5, 0), (7, 0)}
    for bi in range(int(math.log2(n))):
        for li in range(bi + 1):
            m = 2 ** (bi - li)
            out.append((bi, li, m, bi + 1, (bi, li) not in noswap))
    return out


LAYERS = _layer_structure()
L = len(LAYERS)  # 36
QW = L * 128     # q_all free width


def emit(tc, v_in, x_out, n_layers=L):
    nc = tc.nc
    O = mybir.AluOpType
    A = mybir.ActivationFunctionType
    with ExitStack() as ctx:
        pool = ctx.enter_context(tc.tile_pool(name="main", bufs=1))
        scratch = ctx.enter_context(tc.tile_pool(name="scr", bufs=1))
        bfly = ctx.enter_context(tc.tile_pool(name="bfly", bufs=2))

        # ---- load vectors: v_loc = own half, v_oth = other half ----
        # partition p = jh*64 + b
        v_loc = pool.tile([128, 128], FP, tag="vl")
        v_oth = pool.tile([128, 128], FP, tag="vo")
        v_loc2 = pool.tile([128, 128], FP, tag="vl2")
        v_oth2 = pool.tile([128, 128], FP, tag="vo2")
        nc.sync.dma_start(v_loc[0:64, :], v_in[:, 0:128])
        nc.sync.dma_start(v_loc[64:128, :], v_in[:, 128:256])
        nc.sync.dma_start(v_oth[0:64, :], v_in[:, 128:256])
        nc.sync.dma_start(v_oth[64:128, :], v_in[:, 0:128])

        # ---- init X = identity (in (ir, jl) coords the diagonal is uniform:
        # nonzero at ir == jl for every partition) ----
        x = pool.tile([128, XF], HP, tag="x")
        nc.scalar.memzero(x[:])
        diag = bass.AP(x[:].tensor, 0, [[XF, 128], [129, 128]])
        nc.vector.memset(diag, 1.0)

        # ---- q_all[p, t*128 + k]: k = local pair index ----
        q_all = pool.tile([128, QW], HP, tag="qa")
        vl_cur, vl_nxt = v_loc, v_loc2
        vo_cur, vo_nxt = v_oth, v_oth2
        M = 0
        Q_CHUNK = 6
        for t0 in range(0, n_layers, Q_CHUNK):
            tn = min(Q_CHUNK, n_layers - t0)
            cw = tn * 128
            z = scratch.tile([128, Q_CHUNK * 128], FP, tag="z")
            t1 = scratch.tile([128, Q_CHUNK * 128], FP, tag="t1")
            t2 = scratch.tile([128, Q_CHUNK * 128], FP, tag="t2")
            t3 = scratch.tile([128, Q_CHUNK * 128], FP, tag="t3")
            mk = scratch.tile([128, Q_CHUNK * 128], mybir.dt.uint8, tag="mk")
            for ti in range(tn):
                t = t0 + ti
                bi, li, m, fb, swap = LAYERS[t]
                lm = bi - li
                Mlm = (M >> lm) & 1
                if (bi, li) == (7, 0):
                    # pairs (cl, cl+128); dv = v_cur[cl+128] - v_cur[cl]
                    dv0 = z[0:64, ti * 128 : ti * 128 + 128]
                    dv1 = z[64:128, ti * 128 : ti * 128 + 128]
                    nc.vector.tensor_tensor(dv0, vo_cur[0:64, :], vl_cur[0:64, :], O.subtract)
                    nc.vector.tensor_tensor(dv1, vl_cur[64:128, :], vo_cur[64:128, :], O.subtract)
                    continue
                ngrp = 64 // m
                vv = vl_cur[:].rearrange("p (g r j) -> p g r j", g=ngrp, r=2)
                vlo = vv[:, :, Mlm, :]
                vhi = vv[:, :, 1 - Mlm, :]
                dv_t = z[:, ti * 128 : ti * 128 + 64].rearrange(
                    "p (g j) -> p g j", g=ngrp
                )
                if bi <= 5:
                    # flag bit = bit li of local group index, XOR M_fb
                    go = 2 ** li
                    Mfb = (M >> fb) & 1
                    ga = dv_t.rearrange("p (a f g) j -> p a f g j", f=2, g=go)
                    vl_ = vlo.rearrange("p (a f g) j -> p a f g j", f=2, g=go)
                    vh_ = vhi.rearrange("p (a f g) j -> p a f g j", f=2, g=go)
                    asc, dsc = Mfb, 1 - Mfb
                    nc.vector.tensor_tensor(
                        ga[:, :, asc], vh_[:, :, asc], vl_[:, :, asc], O.subtract
                    )
                    nc.vector.tensor_tensor(
                        ga[:, :, dsc], vl_[:, :, dsc], vh_[:, :, dsc], O.subtract
                    )
                elif bi == 6:
                    # flag = jh (partition-half); M bit 7 is always 0
                    vv0 = vl_cur[0:64, :].rearrange("p (g r j) -> p g r j", g=ngrp, r=2)
                    vv1 = vl_cur[64:128, :].rearrange("p (g r j) -> p g r j", g=ngrp, r=2)
                    d0 = z[0:64, ti * 128 : ti * 128 + 64].rearrange("p (g j) -> p g j", g=ngrp)
                    d1 = z[64:128, ti * 128 : ti * 128 + 64].rearrange("p (g j) -> p g j", g=ngrp)
                    nc.vector.tensor_tensor(d0, vv0[:, :, 1 - Mlm, :], vv0[:, :, Mlm, :], O.subtract)
                    nc.vector.tensor_tensor(d1, vv1[:, :, Mlm, :], vv1[:, :, 1 - Mlm, :], O.subtract)
                else:
                    # bi == 7, m <= 64: no flag
                    nc.vector.tensor_tensor(dv_t, vhi, vlo, O.subtract)
                if swap:
                    # v_nxt[c] = v_cur[c ^ m] (local, both tiles)
                    for cur, nxt in ((vl_cur, vl_nxt), (vo_cur, vo_nxt)):
                        vv_ = cur[:].rearrange("p (g r j) -> p g r j", g=ngrp, r=2)
                        vn_ = nxt[:].rearrange("p (g r j) -> p g r j", g=ngrp, r=2)
                        nc.scalar.copy(vn_[:, :, 0, :], vv_[:, :, 1, :])
                        nc.scalar.copy(vn_[:, :, 1, :], vv_[:, :, 0, :])
                    vl_cur, vl_nxt = vl_nxt, vl_cur
                    vo_cur, vo_nxt = vo_nxt, vo_cur
                    M ^= m

            # q = arctan(10*z)/pi + 0.5, range-reduced:
            #   |10z| < 1 : arctan(10z);  else sign(z)*(pi/2 - arctan(1/|10z|))
            qc = q_all[:, t0 * 128 : t0 * 128 + cw]
            qf = scratch.tile([128, Q_CHUNK * 128], FP, tag="qf")
            qfc = qf[:, :cw]
            zc, t1c, t2c, t3c = z[:, :cw], t1[:, :cw], t2[:, :cw], t3[:, :cw]
            nc.vector.tensor_scalar(t1c, zc, -0.1, 0.1, O.max, O.min)
            nc.scalar.activation(qfc, t1c, A.Arctan, bias=0.0, scale=STEEP)
            nc.scalar.activation(t1c, zc, A.Abs, bias=0.0, scale=STEEP)
            nc.gpsimd.tensor_scalar(t2c, t1c, 1.0, None, O.max)
            nc.scalar.activation(t2c, t2c, A.Ln)
            nc.scalar.activation(t2c, t2c, A.Exp, bias=0.0, scale=-1.0)
            nc.scalar.activation(t2c, t2c, A.Arctan)
            nc.scalar.activation(t3c, zc, A.Sign)
            nc.gpsimd.tensor_tensor(t2c, t3c, t2c, O.mult)
            nc.vector.scalar_tensor_tensor(
                t2c, t3c, float(math.pi / 2), t2c, O.mult, O.subtract
            )
            nc.gpsimd.tensor_scalar(mk[:, :cw], t1c, 1.0, None, O.is_lt)
            nc.vector.copy_predicated(t2c, mk[:, :cw], qfc)
            nc.scalar.activation(qc, t2c, A.Copy,
                                 bias=0.5, scale=float(1.0 / math.pi))

        xt = x[:].tensor
        qt = q_all[:].tensor

        def _mk(tensor, off, pdim, dims):
            dims = [d for d in dims if d[1] != 1]
            assert len(dims) <= 3, dims
            return bass.AP(tensor, off, [pdim] + dims)

        # ---- butterfly layers ----
        M = 0
        for t in range(n_layers):
            bi, li, m, fb, swap = LAYERS[t]
            w = 2 ** (bi + 1)
            lm = bi - li
            Mlm = (M >> lm) & 1
            rL, rH = Mlm, 1 - Mlm

            if (bi, li) == (7, 0):
                # pairs (jl, jl+128) across partition halves; before this
                # layer x is nonzero only for ir < 128 on every partition.
                # newLo = H + q*(L-H) at lo slot, newHi = L - q*(L-H) at hi.
                # On i<128 rows (H=0): newLo = q*L, newHi = L - q*L.
                # On i>=128 rows (L=0): newLo = H - q*H, newHi = q*H.
                for c0 in (0, 64):  # ir chunk
                    coff = c0 * 128
                    dims = [[128, 64], [1, 128]]
                    xlo = _mk(xt, coff, [XF, 64], dims)                    # L, p0:64
                    xhi = _mk(xt, 64 * XF + coff, [XF, 64], dims)          # H, p64:
                    xlo_up = _mk(xt, 128 * 128 + coff, [XF, 64], dims)     # lo slot, i>=128
                    xhi_up = _mk(xt, 64 * XF + 128 * 128 + coff, [XF, 64], dims)
                    q0 = _mk(qt, t * 128, [QW, 64], [[0, 64], [1, 128]])
                    q1 = _mk(qt, 64 * QW + t * 128, [QW, 64], [[0, 64], [1, 128]])
                    qd = bfly.tile([128, FD_CAP], HP, tag="qd")
                    qdt = qd[:].tensor
                    sdims = [[128, 64], [1, 128]]
                    qdl = _mk(qdt, 0, [FD_CAP, 64], sdims)
                    qdh = _mk(qdt, 64 * FD_CAP, [FD_CAP, 64], sdims)
                    nc.vector.tensor_tensor(qdl, xlo, q0, O.mult)      # q*L
                    nc.vector.tensor_tensor(xhi_up, xlo, qdl, O.subtract)  # newHi(i<128)
                    nc.vector.tensor_copy(xlo, qdl)                    # newLo(i<128)
                    nc.vector.tensor_tensor(qdh, xhi, q1, O.mult)      # q*H
                    nc.vector.tensor_tensor(xlo_up, xhi, qdh, O.subtract)  # newLo(i>=128)
                    nc.vector.tensor_copy(xhi, qdh)                    # newHi(i>=128)
                continue

            if w <= 128:
                nW, ce, G = 128 // w, w, w // (2 * m)
            else:
                nW, ce, G = 1, 128, 64 // m

            # split plan: HW allows <=3 free dims
            nontriv = sum(1 for c in (nW, ce, G, m) if c > 1)
            if nontriv > 3:
                if G <= nW:
                    g_splits = [(g, 1) for g in range(G)]
                    w_splits = [(0, nW)]
                else:
                    g_splits = [(0, G)]
                    w_splits = [(W, 1) for W in range(nW)]
            else:
                g_splits = [(0, G)]
                w_splits = [(0, nW)]

            i_chunks = [(0, ce)] if w <= 128 else [(0, 128), (128, 128)]

            for (W0, nWi) in w_splits:
                for (g0, Gi) in g_splits:
                    for (c0, cei) in i_chunks:
                        def x_ap(r):
                            off = (W0 * w * 129 + c0 * 128 + g0 * 2 * m
                                   + r * m)
                            dims = [[w * 129, nWi], [128, cei],
                                    [2 * m, Gi], [1, m]]
                            return _mk(xt, off, [XF, 128], dims)

                        def s_ap(tile_h):
                            st = tile_h[:].tensor
                            dims = [[cei * Gi * m, nWi], [Gi * m, cei],
                                    [m, Gi], [1, m]]
                            return _mk(st, 0, [FD_CAP, 128], dims)

                        qoff = t * 128 + W0 * (w // 2) + g0 * m
                        qdims = [[w // 2, nWi], [0, cei], [m, Gi], [1, m]]
                        qb = _mk(qt, qoff, [QW, 128], qdims)

                        sL = x_ap(rL)
                        sH = x_ap(rH)
                        d = bfly.tile([128, FD_CAP], HP, tag="d")
                        qd = bfly.tile([128, FD_CAP], HP, tag="qd")
                        dp = s_ap(d)
                        qdp = s_ap(qd)
                        nc.vector.tensor_tensor(dp, sL, sH, O.subtract)
                        nc.vector.tensor_tensor(qdp, dp, qb, O.mult)
                        if swap:
                            nc.vector.tensor_tensor(sH, sH, qdp, O.add)
                            nc.vector.tensor_tensor(sL, sL, qdp, O.subtract)
                        else:
                            d2 = bfly.tile([128, FD_CAP // 2], HP, tag="d2")
                            d2p = _mk(d2[:].tensor, 0, [FD_CAP // 2, 128],
                                      [[cei * Gi * m, nWi], [Gi * m, cei],
                                       [m, Gi], [1, m]])
                            nc.vector.tensor_tensor(d2p, dp, qdp, O.subtract)
                            nc.vector.tensor_tensor(sL, sH, qdp, O.add)
                            nc.vector.tensor_tensor(sH, sH, d2p, O.add)
            if swap:
                M ^= m
        assert n_layers < L or M == 0, f"final XOR mask {M} != 0"

        # ---- write out ----
        # x_out dram [2, 64, 256, 128] = (jh, b, i, jl); jh=1 needs ir-halves
        # swapped back (global i = ir ^ 128).
        o0 = x_out[0].rearrange("b i j -> b (i j)")
        nc.sync.dma_start(o0, x[0:64, :])
        o1 = x_out[1]
        o1lo = o1[:, 0:128].rearrange("b i j -> b (i j)")
        o1hi = o1[:, 128:256].rearrange("b i j -> b (i j)")
        nc.sync.dma_start(o1lo, x[64:128, 128 * 128 : XF])
        nc.sync.dma_start(o1hi, x[64:128, 0 : 128 * 128])


def build_nc(n_layers=L):
    nc = bacc.Bacc("TRN2", target_bir_lowering=False, debug=False)
    v_in = nc.declare_dram_parameter("vectors", [B_LOC, N], FP, isOutput=False)
    x_out = nc.declare_dram_parameter("out", [2, B_LOC, N, 128], HP, isOutput=True)
    with tile.TileContext(nc) as tc:
        emit(tc, v_in[:], x_out[:], n_layers=n_layers)
    nc.finalize()
    return nc


_NC_CACHE = {}


def kernel(**inputs) -> np.ndarray:
    vectors = np.asarray(inputs["vectors"], dtype=np.float32)
    assert vectors.shape == (B_FULL, N)
    if "default" not in _NC_CACHE:
        _NC_CACHE["default"] = build_nc()
    nc = _NC_CACHE["default"]
    in_maps = [
        {"vectors": vectors[c * B_LOC : (c + 1) * B_LOC]} for c in range(N_CORES)
    ]
    res = run_bass_kernel_spmd(nc, in_maps, core_ids=list(range(N_CORES)))
    outs = []
    for c in range(N_CORES):
        o = np.asarray(res.results[c]["out"])  # [2, 64, 256, 128] fp16
        outs.append(
            np.transpose(o, (1, 2, 0, 3)).reshape(B_LOC, N, N).astype(np.float32)
        )
    return np.concatenate(outs, axis=0)


if __name__ == "__main__":
    rng = np.random.default_rng(0)
    v = rng.normal(size=(B_FULL, N)).astype(np.float32)
    o = kernel(vectors=v)
    print("kernel output shape:", o.shape, o.dtype)


# revision 9
# speedup vs baseline: 2.6793x; 1.1987x over previous
"""Trainium2 Bass kernel for DiffSortNet (differentiable bitonic sort network).

Full inputs in, full outputs out. Pure data parallel over 8 NeuronCores
(batch 512 -> 64 per core). Selector matrices are compile-time constants of
the bitonic network for n=256; the kernel derives (lo, hi, direction) itself.

Math (per batch b, layer with pair distance m):
    pairs (lo, hi=lo+m), direction flag = bit_{block+1}(lo)
    dv = (v[hi]-v[lo]) * (flag ? -1 : +1);  q = arctan(10*dv)/pi + 0.5
    X[:,lo], X[:,hi] = H + q*(L-H), L - q*(L-H)      (L/H = old X cols)

Performance structure (v2 vs the old h-split kernel):
  * layout: partition p = jh*64 + b  (jh = column half of X, b = batch).
    Free dim = (ir, jl): jl = j & 127, and ir = i XOR (jh*128) -- the i halves
    are swapped on the jh=1 partitions so every window's free-dim offsets are
    partition-uniform. All 35 within-half layers then run on all 128
    partitions in a single instruction stream (the old kernel issued each op
    twice on 64-partition halves).
  * fp16 X and q: DVE's 2x_1p fast mode (2-byte dtype + packed innermost AP)
    doubles tensor_tensor throughput for every m>=2 butterfly op.
  * window sparsity: after block bi, column j of X is supported only on
    rows i inside the aligned 2^(bi+1)-window of j.
  * swapped in-place writes (newLo at hi slot) avoid a 5th pass; 4 layers
    use the 5-pass non-swap form so the XOR bookkeeping cancels (M_final=0).
  * layer (7,0) (pairs span column halves) exploits disjoint supports
    (L lives on i<128, H on i>=128) to run as 4 mults/subs + 2 copies.
"""
import math
import sys
from contextlib import ExitStack

sys.path.insert(0, "/opt/trn_rl_repo")

import numpy as np

import concourse.bacc as bacc
import concourse.bass as bass
import concourse.mybir as mybir
import concourse.tile as tile
from concourse.bass_utils import run_bass_kernel_spmd

N = 256
B_FULL = 512
N_CORES = 8
B_LOC = B_FULL // N_CORES  # 64
STEEP = 10.0
FP = mybir.dt.float32
HP = mybir.dt.float16
LOG2N = 8
XF = 256 * 128  # x free size per partition: (ir, jl)
FD_CAP = 8192   # max free elements per butterfly instruction


def _layer_structure(n=N):
    out = []
    noswap = {(1, 0), (3, 0), (5, 0), (7, 0)}
    for bi in range(int(math.log2(n))):
        for li in range(bi + 1):
            m = 2 ** (bi - li)
            out.append((bi, li, m, bi + 1, (bi, li) not in noswap))
    return out


LAYERS = _layer_structure()
L = len(LAYERS)  # 36
QW = L * 128     # q_all free width


def emit(tc, v_in, x_out, n_layers=L):
    nc = tc.nc
    O = mybir.AluOpType
    A = mybir.ActivationFunctionType
    with ExitStack() as ctx:
        pool = ctx.enter_context(tc.tile_pool(name="main", bufs=1))
        scratch = ctx.enter_context(tc.tile_pool(name="scr", bufs=2))
        bfly = ctx.enter_context(tc.tile_pool(name="bfly", bufs=2))

        # ---- load vectors: v_loc = own half, v_oth = other half ----
        # partition p = jh*64 + b
        v_loc = pool.tile([128, 128], FP, tag="vl")
        v_oth = pool.tile([128, 128], FP, tag="vo")
        v_loc2 = pool.tile([128, 128], FP, tag="vl2")
        v_oth2 = pool.tile([128, 128], FP, tag="vo2")
        nc.sync.dma_start(v_loc[0:64, :], v_in[:, 0:128])
        nc.sync.dma_start(v_loc[64:128, :], v_in[:, 128:256])
        nc.sync.dma_start(v_oth[0:64, :], v_in[:, 128:256])
        nc.sync.dma_start(v_oth[64:128, :], v_in[:, 0:128])

        # ---- init X = identity (in (ir, jl) coords the diagonal is uniform:
        # nonzero at ir == jl for every partition) ----
        x = pool.tile([128, XF], HP, tag="x")
        nc.vector.memset(x[:], 0.0)
        diag = bass.AP(x[:].tensor, 0, [[XF, 128], [129, 128]])
        nc.vector.memset(diag, 1.0)

        # ---- q_all[p, t*128 + k]: k = local pair index ----
        q_all = pool.tile([128, QW], HP, tag="qa")
        qstate = dict(vl=(v_loc, v_loc2), vo=(v_oth, v_oth2), M=0)
        Q_CHUNK = 6

        def emit_q_chunk(t0):
            tn = min(Q_CHUNK, n_layers - t0)
            cw = tn * 128
            vl_cur, vl_nxt = qstate["vl"]
            vo_cur, vo_nxt = qstate["vo"]
            M = qstate["M"]
            z = scratch.tile([128, Q_CHUNK * 128], FP, tag="z")
            zq = scratch.tile([128, Q_CHUNK * 128], FP, tag="zq")
            for ti in range(tn):
                t = t0 + ti
                bi, li, m, fb, swap = LAYERS[t]
                lm = bi - li
                Mlm = (M >> lm) & 1
                if (bi, li) == (7, 0):
                    # pairs (cl, cl+128); dv = v_cur[cl+128] - v_cur[cl]
                    dv0 = z[0:64, ti * 128 : ti * 128 + 128]
                    dv1 = z[64:128, ti * 128 : ti * 128 + 128]
                    nc.vector.tensor_tensor(dv0, vo_cur[0:64, :], vl_cur[0:64, :], O.subtract)
                    nc.vector.tensor_tensor(dv1, vl_cur[64:128, :], vo_cur[64:128, :], O.subtract)
                    continue
                ngrp = 64 // m
                vv = vl_cur[:].rearrange("p (g r j) -> p g r j", g=ngrp, r=2)
                vlo = vv[:, :, Mlm, :]
                vhi = vv[:, :, 1 - Mlm, :]
                dv_t = z[:, ti * 128 : ti * 128 + 64].rearrange(
                    "p (g j) -> p g j", g=ngrp
                )
                if bi <= 5:
                    # flag bit = bit li of local group index, XOR M_fb
                    go = 2 ** li
                    Mfb = (M >> fb) & 1
                    ga = dv_t.rearrange("p (a f g) j -> p a f g j", f=2, g=go)
                    vl_ = vlo.rearrange("p (a f g) j -> p a f g j", f=2, g=go)
                    vh_ = vhi.rearrange("p (a f g) j -> p a f g j", f=2, g=go)
                    asc, dsc = Mfb, 1 - Mfb
                    nc.vector.tensor_tensor(
                        ga[:, :, asc], vh_[:, :, asc], vl_[:, :, asc], O.subtract
                    )
                    nc.vector.tensor_tensor(
                        ga[:, :, dsc], vl_[:, :, dsc], vh_[:, :, dsc], O.subtract
                    )
                elif bi == 6:
                    # flag = jh (partition-half); M bit 7 is always 0
                    vv0 = vl_cur[0:64, :].rearrange("p (g r j) -> p g r j", g=ngrp, r=2)
                    vv1 = vl_cur[64:128, :].rearrange("p (g r j) -> p g r j", g=ngrp, r=2)
                    d0 = z[0:64, ti * 128 : ti * 128 + 64].rearrange("p (g j) -> p g j", g=ngrp)
                    d1 = z[64:128, ti * 128 : ti * 128 + 64].rearrange("p (g j) -> p g j", g=ngrp)
                    nc.vector.tensor_tensor(d0, vv0[:, :, 1 - Mlm, :], vv0[:, :, Mlm, :], O.subtract)
                    nc.vector.tensor_tensor(d1, vv1[:, :, Mlm, :], vv1[:, :, 1 - Mlm, :], O.subtract)
                else:
                    # bi == 7, m <= 64: no flag
                    nc.vector.tensor_tensor(dv_t, vhi, vlo, O.subtract)
                if swap:
                    # v_nxt[c] = v_cur[c ^ m] (local, both tiles)
                    for cur, nxt in ((vl_cur, vl_nxt), (vo_cur, vo_nxt)):
                        vv_ = cur[:].rearrange("p (g r j) -> p g r j", g=ngrp, r=2)
                        vn_ = nxt[:].rearrange("p (g r j) -> p g r j", g=ngrp, r=2)
                        nc.vector.tensor_copy(vn_[:, :, 0, :], vv_[:, :, 1, :])
                        nc.vector.tensor_copy(vn_[:, :, 1, :], vv_[:, :, 0, :])
                    vl_cur, vl_nxt = vl_nxt, vl_cur
                    vo_cur, vo_nxt = vo_nxt, vo_cur
                    M ^= m

            # q = arctan(10*z)/pi + 0.5 (ACT arctan table; then fused
            # scale/bias + fp16 convert on DVE)
            qc = q_all[:, t0 * 128 : t0 * 128 + cw]
            zc, zqc = z[:, :cw], zq[:, :cw]
            nc.scalar.activation(zqc, zc, A.Arctan, bias=0.0, scale=STEEP)
            nc.vector.tensor_scalar(qc, zqc, float(1.0 / math.pi), 0.5,
                                    O.mult, O.add)
            qstate["vl"] = (vl_cur, vl_nxt)
            qstate["vo"] = (vo_cur, vo_nxt)
            qstate["M"] = M

        xt = x[:].tensor
        qt = q_all[:].tensor

        def _mk(tensor, off, pdim, dims):
            dims = [d for d in dims if d[1] != 1]
            assert len(dims) <= 3, dims
            return bass.AP(tensor, off, [pdim] + dims)

        # ---- butterfly layers ----
        bstate = dict(M=0)

        def emit_bfly_layer(t):
            M = bstate["M"]
            bi, li, m, fb, swap = LAYERS[t]
            w = 2 ** (bi + 1)
            lm = bi - li
            Mlm = (M >> lm) & 1
            rL, rH = Mlm, 1 - Mlm

            if (bi, li) == (7, 0):
                # pairs (jl, jl+128) across partition halves; before this
                # layer x is nonzero only for ir < 128 on every partition.
                # newLo = H + q*(L-H) at lo slot, newHi = L - q*(L-H) at hi.
                # On i<128 rows (H=0): newLo = q*L, newHi = L - q*L.
                # On i>=128 rows (L=0): newLo = H - q*H, newHi = q*H.
                for c0 in (0, 64):  # ir chunk
                    coff = c0 * 128
                    dims = [[128, 64], [1, 128]]
                    xlo = _mk(xt, coff, [XF, 64], dims)                    # L, p0:64
                    xhi = _mk(xt, 64 * XF + coff, [XF, 64], dims)          # H, p64:
                    xlo_up = _mk(xt, 128 * 128 + coff, [XF, 64], dims)     # lo slot, i>=128
                    xhi_up = _mk(xt, 64 * XF + 128 * 128 + coff, [XF, 64], dims)
                    q0 = _mk(qt, t * 128, [QW, 64], [[0, 64], [1, 128]])
                    q1 = _mk(qt, 64 * QW + t * 128, [QW, 64], [[0, 64], [1, 128]])
                    qd = bfly.tile([128, FD_CAP], HP, tag="qd")
                    qdt = qd[:].tensor
                    sdims = [[128, 64], [1, 128]]
                    qdl = _mk(qdt, 0, [FD_CAP, 64], sdims)
                    qdh = _mk(qdt, 64 * FD_CAP, [FD_CAP, 64], sdims)
                    nc.vector.tensor_tensor(qdl, xlo, q0, O.mult)      # q*L
                    nc.vector.tensor_tensor(xhi_up, xlo, qdl, O.subtract)  # newHi(i<128)
                    nc.vector.tensor_copy(xlo, qdl)                    # newLo(i<128)
                    nc.vector.tensor_tensor(qdh, xhi, q1, O.mult)      # q*H
                    nc.vector.tensor_tensor(xlo_up, xhi, qdh, O.subtract)  # newLo(i>=128)
                    nc.vector.tensor_copy(xhi, qdh)                    # newHi(i>=128)
                return

            if w <= 128:
                nW, ce, G = 128 // w, w, w // (2 * m)
            else:
                nW, ce, G = 1, 128, 64 // m

            # split plan: HW allows <=3 free dims
            nontriv = sum(1 for c in (nW, ce, G, m) if c > 1)
            if nontriv > 3:
                if G <= nW:
                    g_splits = [(g, 1) for g in range(G)]
                    w_splits = [(0, nW)]
                else:
                    g_splits = [(0, G)]
                    w_splits = [(W, 1) for W in range(nW)]
            else:
                g_splits = [(0, G)]
                w_splits = [(0, nW)]

            i_chunks = [(0, ce)] if w <= 128 else [(0, 128), (128, 128)]

            for (W0, nWi) in w_splits:
                for (g0, Gi) in g_splits:
                    for (c0, cei) in i_chunks:
                        def x_ap(r):
                            off = (W0 * w * 129 + c0 * 128 + g0 * 2 * m
                                   + r * m)
                            dims = [[w * 129, nWi], [128, cei],
                                    [2 * m, Gi], [1, m]]
                            return _mk(xt, off, [XF, 128], dims)

                        def s_ap(tile_h):
                            st = tile_h[:].tensor
                            dims = [[cei * Gi * m, nWi], [Gi * m, cei],
                                    [m, Gi], [1, m]]
                            return _mk(st, 0, [FD_CAP, 128], dims)

                        qoff = t * 128 + W0 * (w // 2) + g0 * m
                        qdims = [[w // 2, nWi], [0, cei], [m, Gi], [1, m]]
                        qb = _mk(qt, qoff, [QW, 128], qdims)

                        sL = x_ap(rL)
                        sH = x_ap(rH)
                        d = bfly.tile([128, FD_CAP], HP, tag="d")
                        qd = bfly.tile([128, FD_CAP], HP, tag="qd")
                        dp = s_ap(d)
                        qdp = s_ap(qd)
                        nc.vector.tensor_tensor(dp, sL, sH, O.subtract)
                        nc.vector.tensor_tensor(qdp, dp, qb, O.mult)
                        if swap:
                            nc.vector.tensor_tensor(sH, sH, qdp, O.add)
                            nc.vector.tensor_tensor(sL, sL, qdp, O.subtract)
                        else:
                            d2 = bfly.tile([128, FD_CAP // 2], HP, tag="d2")
                            d2p = _mk(d2[:].tensor, 0, [FD_CAP // 2, 128],
                                      [[cei * Gi * m, nWi], [Gi * m, cei],
                                       [m, Gi], [1, m]])
                            nc.vector.tensor_tensor(d2p, dp, qdp, O.subtract)
                            nc.vector.tensor_tensor(sL, sH, qdp, O.add)
                            nc.vector.tensor_tensor(sH, sH, d2p, O.add)
            if swap:
                bstate["M"] = M ^ m

        # ---- interleaved schedule: q-chunk k+LOOKAHEAD is emitted before
        # the butterflies of chunk k, so ACT computes q ahead of DVE ----
        nchunks = (n_layers + Q_CHUNK - 1) // Q_CHUNK
        LOOKAHEAD = 2
        for k in range(min(LOOKAHEAD, nchunks)):
            emit_q_chunk(k * Q_CHUNK)
        for k in range(nchunks):
            if k + LOOKAHEAD < nchunks:
                emit_q_chunk((k + LOOKAHEAD) * Q_CHUNK)
            for t in range(k * Q_CHUNK, min((k + 1) * Q_CHUNK, n_layers)):
                emit_bfly_layer(t)
        assert n_layers < L or bstate["M"] == 0, f"final XOR mask {bstate['M']}"

        # ---- write out (split by ir-halves so DMA overlaps the last layer) ----
        # x_out dram [2, 64, 256, 128] = (jh, b, i, jl); jh=1 needs ir-halves
        # swapped back (global i = ir ^ 128).
        o0 = x_out[0]
        o0lo = o0[:, 0:128].rearrange("b i j -> b (i j)")
        o0hi = o0[:, 128:256].rearrange("b i j -> b (i j)")
        nc.sync.dma_start(o0lo, x[0:64, 0 : 128 * 128])
        nc.sync.dma_start(o0hi, x[0:64, 128 * 128 : XF])
        o1 = x_out[1]
        o1lo = o1[:, 0:128].rearrange("b i j -> b (i j)")
        o1hi = o1[:, 128:256].rearrange("b i j -> b (i j)")
        nc.sync.dma_start(o1lo, x[64:128, 128 * 128 : XF])
        nc.sync.dma_start(o1hi, x[64:128, 0 : 128 * 128])


def build_nc(n_layers=L):
    nc = bacc.Bacc("TRN2", target_bir_lowering=False, debug=False)
    v_in = nc.declare_dram_parameter("vectors", [B_LOC, N], FP, isOutput=False)
    x_out = nc.declare_dram_parameter("out", [2, B_LOC, N, 128], HP, isOutput=True)
    with tile.TileContext(nc) as tc:
        emit(tc, v_in[:], x_out[:], n_layers=n_layers)
    nc.finalize()
    return nc


_NC_CACHE = {}


def kernel(**inputs) -> np.ndarray:
    vectors = np.asarray(inputs["vectors"], dtype=np.float32)
    assert vectors.shape == (B_FULL, N)
    if "default" not in _NC_CACHE:
        _NC_CACHE["default"] = build_nc()
    nc = _NC_CACHE["default"]
    in_maps = [
        {"vectors": vectors[c * B_LOC : (c + 1) * B_LOC]} for c in range(N_CORES)
    ]
    res = run_bass_kernel_spmd(nc, in_maps, core_ids=list(range(N_CORES)))
    outs = []
    for c in range(N_CORES):
        o = np.asarray(res.results[c]["out"])  # [2, 64, 256, 128] fp16
        outs.append(
            np.transpose(o, (1, 2, 0, 3)).reshape(B_LOC, N, N).astype(np.float32)
        )
    return np.concatenate(outs, axis=0)


if __name__ == "__main__":
    rng = np.random.default_rng(0)
    v = rng.normal(size=(B_FULL, N)).astype(np.float32)
    o = kernel(vectors=v)
    print("kernel output shape:", o.shape, o.dtype)


# revision 14
# speedup vs baseline: 2.7894x; 1.0411x over previous
"""Trainium2 Bass kernel for DiffSortNet (differentiable bitonic sort network).

Full inputs in, full outputs out. Pure data parallel over 8 NeuronCores
(batch 512 -> 64 per core). Selector matrices are compile-time constants of
the bitonic network for n=256; the kernel derives (lo, hi, direction) itself.

Math (per batch b, layer with pair distance m):
    pairs (lo, hi=lo+m), direction flag = bit_{block+1}(lo)
    dv = (v[hi]-v[lo]) * (flag ? -1 : +1);  q = arctan(10*dv)/pi + 0.5
    X[:,lo], X[:,hi] = H + q*(L-H), L - q*(L-H)      (L/H = old X cols)

Performance structure (v2 vs the old h-split kernel):
  * layout: partition p = jh*64 + b  (jh = column half of X, b = batch).
    Free dim = (ir, jl): jl = j & 127, and ir = i XOR (jh*128) -- the i halves
    are swapped on the jh=1 partitions so every window's free-dim offsets are
    partition-uniform. All 35 within-half layers then run on all 128
    partitions in a single instruction stream (the old kernel issued each op
    twice on 64-partition halves).
  * fp16 X and q: DVE's 2x_1p fast mode (2-byte dtype + packed innermost AP)
    doubles tensor_tensor throughput for every m>=2 butterfly op.
  * window sparsity: after block bi, column j of X is supported only on
    rows i inside the aligned 2^(bi+1)-window of j.
  * swapped in-place writes (newLo at hi slot) avoid a 5th pass; 4 layers
    use the 5-pass non-swap form so the XOR bookkeeping cancels (M_final=0).
  * layer (7,0) (pairs span column halves) exploits disjoint supports
    (L lives on i<128, H on i>=128) to run as 4 mults/subs + 2 copies.
"""
import math
import sys
from contextlib import ExitStack

sys.path.insert(0, "/opt/trn_rl_repo")

import numpy as np

import concourse.bacc as bacc
import concourse.bass as bass
import concourse.mybir as mybir
import concourse.tile as tile
from concourse.bass_utils import run_bass_kernel_spmd

N = 256
B_FULL = 512
N_CORES = 8
B_LOC = B_FULL // N_CORES  # 64
STEEP = 10.0
FP = mybir.dt.float32
HP = mybir.dt.float16
LOG2N = 8
XF = 256 * 128  # x free size per partition: (ir, jl)
FD_CAP = 8192   # max free elements per butterfly instruction


def _layer_structure(n=N):
    out = []
    noswap = {(1, 0), (3, 0), (5, 0), (7, 0)}
    for bi in range(int(math.log2(n))):
        for li in range(bi + 1):
            m = 2 ** (bi - li)
            out.append((bi, li, m, bi + 1, (bi, li) not in noswap))
    return out


LAYERS = _layer_structure()
L = len(LAYERS)  # 36
QW = L * 128     # q_all free width


def emit(tc, v_in, x_out, n_layers=L):
    nc = tc.nc
    O = mybir.AluOpType
    A = mybir.ActivationFunctionType
    with ExitStack() as ctx:
        pool = ctx.enter_context(tc.tile_pool(name="main", bufs=1))
        scratch = ctx.enter_context(tc.tile_pool(name="scr", bufs=2))
        bfly = ctx.enter_context(tc.tile_pool(name="bfly", bufs=2))

        # ---- load vectors: v_loc = own half, v_oth = other half ----
        # partition p = jh*64 + b
        v_loc = pool.tile([128, 128], FP, tag="vl")
        v_oth = pool.tile([128, 128], FP, tag="vo")
        v_loc2 = pool.tile([128, 128], FP, tag="vl2")
        v_oth2 = pool.tile([128, 128], FP, tag="vo2")
        nc.sync.dma_start(v_loc[0:64, :], v_in[:, 0:128])
        nc.sync.dma_start(v_loc[64:128, :], v_in[:, 128:256])
        nc.sync.dma_start(v_oth[0:64, :], v_in[:, 128:256])
        nc.sync.dma_start(v_oth[64:128, :], v_in[:, 0:128])

        # ---- init X = identity (in (ir, jl) coords the diagonal is uniform:
        # nonzero at ir == jl for every partition) ----
        # only ir<128 is ever read before written ((7,0) fully writes ir>=128),
        # and a uint32 view halves memset element count again
        x = pool.tile([128, XF], HP, tag="x")
        xz = bass.AP(x[:].tensor, 0, [[XF, 128], [1, 128 * 128]]).bitcast(
            mybir.dt.uint32
        )
        nc.vector.memset(xz, 0)
        diag = bass.AP(x[:].tensor, 0, [[XF, 128], [129, 128]])
        nc.vector.memset(diag, 1.0)

        # ---- q_all[p, t*128 + k]: k = local pair index ----
        q_all = pool.tile([128, QW], HP, tag="qa")
        qstate = dict(vl=(v_loc, v_loc2), vo=(v_oth, v_oth2), M=0)
        Q_CHUNK = 6

        def emit_q_chunk(t0):
            tn = min(Q_CHUNK, n_layers - t0)
            cw = tn * 128
            vl_cur, vl_nxt = qstate["vl"]
            vo_cur, vo_nxt = qstate["vo"]
            M = qstate["M"]
            z = scratch.tile([128, Q_CHUNK * 128], FP, tag="z")
            zq = scratch.tile([128, Q_CHUNK * 128], FP, tag="zq")
            for ti in range(tn):
                t = t0 + ti
                bi, li, m, fb, swap = LAYERS[t]
                lm = bi - li
                Mlm = (M >> lm) & 1
                if (bi, li) == (7, 0):
                    # pairs (cl, cl+128); dv = v_cur[cl+128] - v_cur[cl]
                    dv0 = z[0:64, ti * 128 : ti * 128 + 128]
                    dv1 = z[64:128, ti * 128 : ti * 128 + 128]
                    nc.vector.tensor_tensor(dv0, vo_cur[0:64, :], vl_cur[0:64, :], O.subtract)
                    nc.vector.tensor_tensor(dv1, vl_cur[64:128, :], vo_cur[64:128, :], O.subtract)
                    continue
                ngrp = 64 // m
                vv = vl_cur[:].rearrange("p (g r j) -> p g r j", g=ngrp, r=2)
                vlo = vv[:, :, Mlm, :]
                vhi = vv[:, :, 1 - Mlm, :]
                dv_t = z[:, ti * 128 : ti * 128 + 64].rearrange(
                    "p (g j) -> p g j", g=ngrp
                )
                if bi <= 5:
                    # flag bit = bit li of local group index, XOR M_fb
                    go = 2 ** li
                    Mfb = (M >> fb) & 1
                    ga = dv_t.rearrange("p (a f g) j -> p a f g j", f=2, g=go)
                    vl_ = vlo.rearrange("p (a f g) j -> p a f g j", f=2, g=go)
                    vh_ = vhi.rearrange("p (a f g) j -> p a f g j", f=2, g=go)
                    asc, dsc = Mfb, 1 - Mfb
                    nc.vector.tensor_tensor(
                        ga[:, :, asc], vh_[:, :, asc], vl_[:, :, asc], O.subtract
                    )
                    nc.vector.tensor_tensor(
                        ga[:, :, dsc], vl_[:, :, dsc], vh_[:, :, dsc], O.subtract
                    )
                elif bi == 6:
                    # flag = jh (partition-half); M bit 7 is always 0
                    vv0 = vl_cur[0:64, :].rearrange("p (g r j) -> p g r j", g=ngrp, r=2)
                    vv1 = vl_cur[64:128, :].rearrange("p (g r j) -> p g r j", g=ngrp, r=2)
                    d0 = z[0:64, ti * 128 : ti * 128 + 64].rearrange("p (g j) -> p g j", g=ngrp)
                    d1 = z[64:128, ti * 128 : ti * 128 + 64].rearrange("p (g j) -> p g j", g=ngrp)
                    nc.vector.tensor_tensor(d0, vv0[:, :, 1 - Mlm, :], vv0[:, :, Mlm, :], O.subtract)
                    nc.vector.tensor_tensor(d1, vv1[:, :, Mlm, :], vv1[:, :, 1 - Mlm, :], O.subtract)
                else:
                    # bi == 7, m <= 64: no flag
                    nc.vector.tensor_tensor(dv_t, vhi, vlo, O.subtract)
                if swap:
                    # v_nxt[c] = v_cur[c ^ m] (local, both tiles)
                    for cur, nxt in ((vl_cur, vl_nxt), (vo_cur, vo_nxt)):
                        vv_ = cur[:].rearrange("p (g r j) -> p g r j", g=ngrp, r=2)
                        vn_ = nxt[:].rearrange("p (g r j) -> p g r j", g=ngrp, r=2)
                        nc.vector.tensor_copy(vn_[:, :, 0, :], vv_[:, :, 1, :])
                        nc.vector.tensor_copy(vn_[:, :, 1, :], vv_[:, :, 0, :])
                    vl_cur, vl_nxt = vl_nxt, vl_cur
                    vo_cur, vo_nxt = vo_nxt, vo_cur
                    M ^= m

            # q = arctan(10*z)/pi + 0.5 (ACT arctan table; then fused
            # scale/bias + fp16 convert on DVE)
            qc = q_all[:, t0 * 128 : t0 * 128 + cw]
            zc, zqc = z[:, :cw], zq[:, :cw]
            nc.scalar.activation(zqc, zc, A.Arctan, bias=0.0, scale=STEEP)
            nc.vector.tensor_scalar(qc, zqc, float(1.0 / math.pi), 0.5,
                                    O.mult, O.add)
            qstate["vl"] = (vl_cur, vl_nxt)
            qstate["vo"] = (vo_cur, vo_nxt)
            qstate["M"] = M

        xt = x[:].tensor
        qt = q_all[:].tensor

        def _mk(tensor, off, pdim, dims):
            dims = [d for d in dims if d[1] != 1]
            assert len(dims) <= 3, dims
            return bass.AP(tensor, off, [pdim] + dims)

        # ---- butterfly layers ----
        bstate = dict(M=0)

        def emit_bfly_layer(t, ic=None):
            M = bstate["M"]
            bi, li, m, fb, swap = LAYERS[t]
            w = 2 ** (bi + 1)
            lm = bi - li
            Mlm = (M >> lm) & 1
            rL, rH = Mlm, 1 - Mlm

            if (bi, li) == (7, 0):
                # pairs (jl, jl+128) across partition halves; before this
                # layer x is nonzero only for ir < 128 on every partition.
                # newLo = H + q*(L-H) at lo slot, newHi = L - q*(L-H) at hi.
                # On i<128 rows (H=0): newLo = q*L, newHi = L - q*L.
                # On i>=128 rows (L=0): newLo = H - q*H, newHi = q*H.
                for c0 in (0, 64):  # ir chunk
                    coff = c0 * 128
                    dims = [[128, 64], [1, 128]]
                    xlo = _mk(xt, coff, [XF, 64], dims)                    # L, p0:64
                    xhi = _mk(xt, 64 * XF + coff, [XF, 64], dims)          # H, p64:
                    xlo_up = _mk(xt, 128 * 128 + coff, [XF, 64], dims)     # lo slot, i>=128
                    xhi_up = _mk(xt, 64 * XF + 128 * 128 + coff, [XF, 64], dims)
                    q0 = _mk(qt, t * 128, [QW, 64], [[0, 64], [1, 128]])
                    q1 = _mk(qt, 64 * QW + t * 128, [QW, 64], [[0, 64], [1, 128]])
                    qd = bfly.tile([128, FD_CAP], HP, tag="qd")
                    qdt = qd[:].tensor
                    sdims = [[128, 64], [1, 128]]
                    qdl = _mk(qdt, 0, [FD_CAP, 64], sdims)
                    qdh = _mk(qdt, 64 * FD_CAP, [FD_CAP, 64], sdims)
                    nc.vector.tensor_tensor(qdl, xlo, q0, O.mult)      # q*L
                    nc.vector.tensor_tensor(xhi_up, xlo, qdl, O.subtract)  # newHi(i<128)
                    nc.vector.tensor_copy(xlo, qdl)                    # newLo(i<128)
                    nc.vector.tensor_tensor(qdh, xhi, q1, O.mult)      # q*H
                    nc.vector.tensor_tensor(xlo_up, xhi, qdh, O.subtract)  # newLo(i>=128)
                    nc.vector.tensor_copy(xhi, qdh)                    # newHi(i>=128)
                return

            if w <= 128:
                nW, ce, G = 128 // w, w, w // (2 * m)
            else:
                nW, ce, G = 1, 128, 64 // m

            # split plan: HW allows <=3 free dims
            nontriv = sum(1 for c in (nW, ce, G, m) if c > 1)
            if nontriv > 3:
                if G <= nW:
                    g_splits = [(g, 1) for g in range(G)]
                    w_splits = [(0, nW)]
                else:
                    g_splits = [(0, G)]
                    w_splits = [(W, 1) for W in range(nW)]
            else:
                g_splits = [(0, G)]
                w_splits = [(0, nW)]

            i_chunks = [(0, ce)] if w <= 128 else [(0, 128), (128, 128)]
            if ic is not None:
                i_chunks = [i_chunks[ic]]

            for (W0, nWi) in w_splits:
                for (g0, Gi) in g_splits:
                    for (c0, cei) in i_chunks:
                        def x_ap(r):
                            off = (W0 * w * 129 + c0 * 128 + g0 * 2 * m
                                   + r * m)
                            dims = [[w * 129, nWi], [128, cei],
                                    [2 * m, Gi], [1, m]]
                            return _mk(xt, off, [XF, 128], dims)

                        def s_ap(tile_h):
                            st = tile_h[:].tensor
                            dims = [[cei * Gi * m, nWi], [Gi * m, cei],
                                    [m, Gi], [1, m]]
                            return _mk(st, 0, [FD_CAP, 128], dims)

                        qoff = t * 128 + W0 * (w // 2) + g0 * m
                        qdims = [[w // 2, nWi], [0, cei], [m, Gi], [1, m]]
                        qb = _mk(qt, qoff, [QW, 128], qdims)

                        sL = x_ap(rL)
                        sH = x_ap(rH)
                        d = bfly.tile([128, FD_CAP], HP, tag="d")
                        qd = bfly.tile([128, FD_CAP], HP, tag="qd")
                        dp = s_ap(d)
                        qdp = s_ap(qd)
                        nc.vector.tensor_tensor(dp, sL, sH, O.subtract)
                        nc.vector.tensor_tensor(qdp, dp, qb, O.mult)
                        if swap:
                            nc.vector.tensor_tensor(sH, sH, qdp, O.add)
                            nc.vector.tensor_tensor(sL, sL, qdp, O.subtract)
                        else:
                            d2 = bfly.tile([128, FD_CAP // 2], HP, tag="d2")
                            d2p = _mk(d2[:].tensor, 0, [FD_CAP // 2, 128],
                                      [[cei * Gi * m, nWi], [Gi * m, cei],
                                       [m, Gi], [1, m]])
                            nc.vector.tensor_tensor(d2p, dp, qdp, O.subtract)
                            nc.vector.tensor_tensor(sL, sH, qdp, O.add)
                            nc.vector.tensor_tensor(sH, sH, d2p, O.add)
            if swap and ic in (None, 1):
                bstate["M"] = M ^ m

        # ---- interleaved schedule: q-chunk k+LOOKAHEAD is emitted before
        # the butterflies of chunk k, so ACT computes q ahead of DVE ----
        nchunks = (n_layers + Q_CHUNK - 1) // Q_CHUNK
        LOOKAHEAD = 2
        for k in range(min(LOOKAHEAD, nchunks)):
            emit_q_chunk(k * Q_CHUNK)
        # x_out dram [2, 64, 256, 128] = (jh, b, i, jl); jh=1 needs ir-halves
        # swapped back (global i = ir ^ 128).
        o0 = x_out[0]
        o1 = x_out[1]

        def emit_out_dma(half):
            # half 0: ir<128 rows final -> out[0] i<128, out[1] i>=128
            lo, hi = (0, 128 * 128) if half == 0 else (128 * 128, XF)
            o0s = o0[:, 0:128] if half == 0 else o0[:, 128:256]
            o1s = o1[:, 128:256] if half == 0 else o1[:, 0:128]
            nc.sync.dma_start(o0s.rearrange("b i j -> b (i j)"), x[0:64, lo:hi])
            nc.sync.dma_start(o1s.rearrange("b i j -> b (i j)"), x[64:128, lo:hi])

        last = n_layers - 1
        split_last = n_layers == L  # layer 35 is w=256: split + interleave DMA
        for k in range(nchunks):
            if k + LOOKAHEAD < nchunks:
                emit_q_chunk((k + LOOKAHEAD) * Q_CHUNK)
            for t in range(k * Q_CHUNK, min((k + 1) * Q_CHUNK, n_layers)):
                if t == last and split_last:
                    emit_bfly_layer(t, ic=0)
                    emit_out_dma(0)
                    emit_bfly_layer(t, ic=1)
                    emit_out_dma(1)
                else:
                    emit_bfly_layer(t)
        assert n_layers < L or bstate["M"] == 0, f"final XOR mask {bstate['M']}"
        if not split_last:
            emit_out_dma(0)
            emit_out_dma(1)


def build_nc(n_layers=L):
    nc = bacc.Bacc("TRN2", target_bir_lowering=False, debug=False)
    v_in = nc.declare_dram_parameter("vectors", [B_LOC, N], FP, isOutput=False)
    x_out = nc.declare_dram_parameter("out", [2, B_LOC, N, 128], HP, isOutput=True)
    with tile.TileContext(nc) as tc:
        emit(tc, v_in[:], x_out[:], n_layers=n_layers)
    nc.finalize()
    return nc


_NC_CACHE = {}


def kernel(**inputs) -> np.ndarray:
    vectors = np.asarray(inputs["vectors"], dtype=np.float32)
    assert vectors.shape == (B_FULL, N)
    if "default" not in _NC_CACHE:
        _NC_CACHE["default"] = build_nc()
    nc = _NC_CACHE["default"]
    in_maps = [
        {"vectors": vectors[c * B_LOC : (c + 1) * B_LOC]} for c in range(N_CORES)
    ]
    res = run_bass_kernel_spmd(nc, in_maps, core_ids=list(range(N_CORES)))
    outs = []
    for c in range(N_CORES):
        o = np.asarray(res.results[c]["out"])  # [2, 64, 256, 128] fp16
        outs.append(
            np.transpose(o, (1, 2, 0, 3)).reshape(B_LOC, N, N).astype(np.float32)
        )
    return np.concatenate(outs, axis=0)


if __name__ == "__main__":
    rng = np.random.default_rng(0)
    v = rng.normal(size=(B_FULL, N)).astype(np.float32)
    o = kernel(vectors=v)
    print("kernel output shape:", o.shape, o.dtype)


# revision 15
# speedup vs baseline: 2.8024x; 1.0046x over previous
"""Trainium2 Bass kernel for DiffSortNet (differentiable bitonic sort network).

Full inputs in, full outputs out. Pure data parallel over 8 NeuronCores
(batch 512 -> 64 per core). Selector matrices are compile-time constants of
the bitonic network for n=256; the kernel derives (lo, hi, direction) itself.

Math (per batch b, layer with pair distance m):
    pairs (lo, hi=lo+m), direction flag = bit_{block+1}(lo)
    dv = (v[hi]-v[lo]) * (flag ? -1 : +1);  q = arctan(10*dv)/pi + 0.5
    X[:,lo], X[:,hi] = H + q*(L-H), L - q*(L-H)      (L/H = old X cols)

Performance structure (v2 vs the old h-split kernel):
  * layout: partition p = jh*64 + b  (jh = column half of X, b = batch).
    Free dim = (ir, jl): jl = j & 127, and ir = i XOR (jh*128) -- the i halves
    are swapped on the jh=1 partitions so every window's free-dim offsets are
    partition-uniform. All 35 within-half layers then run on all 128
    partitions in a single instruction stream (the old kernel issued each op
    twice on 64-partition halves).
  * fp16 X and q: DVE's 2x_1p fast mode (2-byte dtype + packed innermost AP)
    doubles tensor_tensor throughput for every m>=2 butterfly op.
  * window sparsity: after block bi, column j of X is supported only on
    rows i inside the aligned 2^(bi+1)-window of j.
  * swapped in-place writes (newLo at hi slot) avoid a 5th pass; 4 layers
    use the 5-pass non-swap form so the XOR bookkeeping cancels (M_final=0).
  * layer (7,0) (pairs span column halves) exploits disjoint supports
    (L lives on i<128, H on i>=128) to run as 4 mults/subs + 2 copies.
"""
import math
import sys
from contextlib import ExitStack

sys.path.insert(0, "/opt/trn_rl_repo")

import numpy as np

import concourse.bacc as bacc
import concourse.bass as bass
import concourse.mybir as mybir
import concourse.tile as tile
from concourse.bass_utils import run_bass_kernel_spmd

N = 256
B_FULL = 512
N_CORES = 8
B_LOC = B_FULL // N_CORES  # 64
STEEP = 10.0
FP = mybir.dt.float32
HP = mybir.dt.float16
LOG2N = 8
XF = 256 * 128  # x free size per partition: (ir, jl)
FD_CAP = 8192   # max free elements per butterfly instruction


def _layer_structure(n=N):
    out = []
    noswap = {(1, 0), (3, 0), (5, 0), (7, 0)}
    for bi in range(int(math.log2(n))):
        for li in range(bi + 1):
            m = 2 ** (bi - li)
            out.append((bi, li, m, bi + 1, (bi, li) not in noswap))
    return out


LAYERS = _layer_structure()
L = len(LAYERS)  # 36
QW = L * 128     # q_all free width


def emit(tc, v_in, x_out, n_layers=L):
    nc = tc.nc
    O = mybir.AluOpType
    A = mybir.ActivationFunctionType
    with ExitStack() as ctx:
        pool = ctx.enter_context(tc.tile_pool(name="main", bufs=1))
        scratch = ctx.enter_context(tc.tile_pool(name="scr", bufs=2))
        bfly = ctx.enter_context(tc.tile_pool(name="bfly", bufs=2))

        # ---- load vectors: v_loc = own half, v_oth = other half ----
        # partition p = jh*64 + b
        vb = pool.tile([128, 256], FP, tag="vb")
        vb2 = pool.tile([128, 256], FP, tag="vb2")
        nc.sync.dma_start(vb[0:64, 0:128], v_in[:, 0:128])
        nc.sync.dma_start(vb[64:128, 0:128], v_in[:, 128:256])
        nc.sync.dma_start(vb[0:64, 128:256], v_in[:, 128:256])
        nc.sync.dma_start(vb[64:128, 128:256], v_in[:, 0:128])

        # ---- init X = identity (in (ir, jl) coords the diagonal is uniform:
        # nonzero at ir == jl for every partition) ----
        # only ir<128 is ever read before written ((7,0) fully writes ir>=128),
        # and a uint32 view halves memset element count again
        x = pool.tile([128, XF], HP, tag="x")
        xz = bass.AP(x[:].tensor, 0, [[XF, 128], [1, 128 * 128]]).bitcast(
            mybir.dt.uint32
        )
        nc.vector.memset(xz, 0)
        diag = bass.AP(x[:].tensor, 0, [[XF, 128], [129, 128]])
        nc.vector.memset(diag, 1.0)

        # ---- q_all[p, t*128 + k]: k = local pair index ----
        q_all = pool.tile([128, QW], HP, tag="qa")
        qstate = dict(v=(vb, vb2), M=0)
        Q_CHUNK = 6

        def emit_q_chunk(t0):
            tn = min(Q_CHUNK, n_layers - t0)
            cw = tn * 128
            v_cur, v_nxt = qstate["v"]
            M = qstate["M"]
            z = scratch.tile([128, Q_CHUNK * 128], FP, tag="z")
            zq = scratch.tile([128, Q_CHUNK * 128], FP, tag="zq")
            for ti in range(tn):
                t = t0 + ti
                bi, li, m, fb, swap = LAYERS[t]
                lm = bi - li
                Mlm = (M >> lm) & 1
                if (bi, li) == (7, 0):
                    # pairs (cl, cl+128); dv = v_cur[cl+128] - v_cur[cl]
                    dv0 = z[0:64, ti * 128 : ti * 128 + 128]
                    dv1 = z[64:128, ti * 128 : ti * 128 + 128]
                    nc.vector.tensor_tensor(dv0, v_cur[0:64, 128:256], v_cur[0:64, 0:128], O.subtract)
                    nc.vector.tensor_tensor(dv1, v_cur[64:128, 0:128], v_cur[64:128, 128:256], O.subtract)
                    continue
                ngrp = 64 // m
                vv = v_cur[:, 0:128].rearrange("p (g r j) -> p g r j", g=ngrp, r=2)
                vlo = vv[:, :, Mlm, :]
                vhi = vv[:, :, 1 - Mlm, :]
                dv_t = z[:, ti * 128 : ti * 128 + 64].rearrange(
                    "p (g j) -> p g j", g=ngrp
                )
                if bi <= 5:
                    # flag bit = bit li of local group index, XOR M_fb
                    go = 2 ** li
                    Mfb = (M >> fb) & 1
                    ga = dv_t.rearrange("p (a f g) j -> p a f g j", f=2, g=go)
                    vl_ = vlo.rearrange("p (a f g) j -> p a f g j", f=2, g=go)
                    vh_ = vhi.rearrange("p (a f g) j -> p a f g j", f=2, g=go)
                    asc, dsc = Mfb, 1 - Mfb
                    nc.vector.tensor_tensor(
                        ga[:, :, asc], vh_[:, :, asc], vl_[:, :, asc], O.subtract
                    )
                    nc.vector.tensor_tensor(
                        ga[:, :, dsc], vl_[:, :, dsc], vh_[:, :, dsc], O.subtract
                    )
                elif bi == 6:
                    # flag = jh (partition-half); M bit 7 is always 0
                    vv0 = v_cur[0:64, 0:128].rearrange("p (g r j) -> p g r j", g=ngrp, r=2)
                    vv1 = v_cur[64:128, 0:128].rearrange("p (g r j) -> p g r j", g=ngrp, r=2)
                    d0 = z[0:64, ti * 128 : ti * 128 + 64].rearrange("p (g j) -> p g j", g=ngrp)
                    d1 = z[64:128, ti * 128 : ti * 128 + 64].rearrange("p (g j) -> p g j", g=ngrp)
                    nc.vector.tensor_tensor(d0, vv0[:, :, 1 - Mlm, :], vv0[:, :, Mlm, :], O.subtract)
                    nc.vector.tensor_tensor(d1, vv1[:, :, Mlm, :], vv1[:, :, 1 - Mlm, :], O.subtract)
                else:
                    # bi == 7, m <= 64: no flag
                    nc.vector.tensor_tensor(dv_t, vhi, vlo, O.subtract)
                if swap:
                    # v_nxt[c] = v_cur[c ^ m] (both halves in one tile: h dim)
                    vv_ = v_cur[:].rearrange("p (h g r j) -> p h g r j", h=2, g=ngrp, r=2)
                    vn_ = v_nxt[:].rearrange("p (h g r j) -> p h g r j", h=2, g=ngrp, r=2)
                    nc.vector.tensor_copy(vn_[:, :, :, 0, :], vv_[:, :, :, 1, :])
                    nc.vector.tensor_copy(vn_[:, :, :, 1, :], vv_[:, :, :, 0, :])
                    v_cur, v_nxt = v_nxt, v_cur
                    M ^= m

            # q = arctan(10*z)/pi + 0.5 (ACT arctan table; then fused
            # scale/bias + fp16 convert on DVE)
            qc = q_all[:, t0 * 128 : t0 * 128 + cw]
            zc, zqc = z[:, :cw], zq[:, :cw]
            nc.scalar.activation(zqc, zc, A.Arctan, bias=0.0, scale=STEEP)
            nc.vector.tensor_scalar(qc, zqc, float(1.0 / math.pi), 0.5,
                                    O.mult, O.add)
            qstate["v"] = (v_cur, v_nxt)
            qstate["M"] = M

        xt = x[:].tensor
        qt = q_all[:].tensor

        def _mk(tensor, off, pdim, dims):
            dims = [d for d in dims if d[1] != 1]
            assert len(dims) <= 3, dims
            return bass.AP(tensor, off, [pdim] + dims)

        # ---- butterfly layers ----
        bstate = dict(M=0)

        def emit_bfly_layer(t, ic=None):
            M = bstate["M"]
            bi, li, m, fb, swap = LAYERS[t]
            w = 2 ** (bi + 1)
            lm = bi - li
            Mlm = (M >> lm) & 1
            rL, rH = Mlm, 1 - Mlm

            if (bi, li) == (7, 0):
                # pairs (jl, jl+128) across partition halves; before this
                # layer x is nonzero only for ir < 128 on every partition.
                # newLo = H + q*(L-H) at lo slot, newHi = L - q*(L-H) at hi.
                # On i<128 rows (H=0): newLo = q*L, newHi = L - q*L.
                # On i>=128 rows (L=0): newLo = H - q*H, newHi = q*H.
                for c0 in (0, 64):  # ir chunk
                    coff = c0 * 128
                    dims = [[128, 64], [1, 128]]
                    xlo = _mk(xt, coff, [XF, 64], dims)                    # L, p0:64
                    xhi = _mk(xt, 64 * XF + coff, [XF, 64], dims)          # H, p64:
                    xlo_up = _mk(xt, 128 * 128 + coff, [XF, 64], dims)     # lo slot, i>=128
                    xhi_up = _mk(xt, 64 * XF + 128 * 128 + coff, [XF, 64], dims)
                    q0 = _mk(qt, t * 128, [QW, 64], [[0, 64], [1, 128]])
                    q1 = _mk(qt, 64 * QW + t * 128, [QW, 64], [[0, 64], [1, 128]])
                    qd = bfly.tile([128, FD_CAP], HP, tag="qd")
                    qdt = qd[:].tensor
                    sdims = [[128, 64], [1, 128]]
                    qdl = _mk(qdt, 0, [FD_CAP, 64], sdims)
                    qdh = _mk(qdt, 64 * FD_CAP, [FD_CAP, 64], sdims)
                    nc.vector.tensor_tensor(qdl, xlo, q0, O.mult)      # q*L
                    nc.vector.tensor_tensor(xhi_up, xlo, qdl, O.subtract)  # newHi(i<128)
                    nc.vector.tensor_copy(xlo, qdl)                    # newLo(i<128)
                    nc.vector.tensor_tensor(qdh, xhi, q1, O.mult)      # q*H
                    nc.vector.tensor_tensor(xlo_up, xhi, qdh, O.subtract)  # newLo(i>=128)
                    nc.vector.tensor_copy(xhi, qdh)                    # newHi(i>=128)
                return

            if w <= 128:
                nW, ce, G = 128 // w, w, w // (2 * m)
            else:
                nW, ce, G = 1, 128, 64 // m

            # split plan: HW allows <=3 free dims
            nontriv = sum(1 for c in (nW, ce, G, m) if c > 1)
            if nontriv > 3:
                if G <= nW:
                    g_splits = [(g, 1) for g in range(G)]
                    w_splits = [(0, nW)]
                else:
                    g_splits = [(0, G)]
                    w_splits = [(W, 1) for W in range(nW)]
            else:
                g_splits = [(0, G)]
                w_splits = [(0, nW)]

            i_chunks = [(0, ce)] if w <= 128 else [(0, 128), (128, 128)]
            if ic is not None:
                i_chunks = [i_chunks[ic]]

            for (W0, nWi) in w_splits:
                for (g0, Gi) in g_splits:
                    for (c0, cei) in i_chunks:
                        def x_ap(r):
                            off = (W0 * w * 129 + c0 * 128 + g0 * 2 * m
                                   + r * m)
                            dims = [[w * 129, nWi], [128, cei],
                                    [2 * m, Gi], [1, m]]
                            return _mk(xt, off, [XF, 128], dims)

                        def s_ap(tile_h):
                            st = tile_h[:].tensor
                            dims = [[cei * Gi * m, nWi], [Gi * m, cei],
                                    [m, Gi], [1, m]]
                            return _mk(st, 0, [FD_CAP, 128], dims)

                        qoff = t * 128 + W0 * (w // 2) + g0 * m
                        qdims = [[w // 2, nWi], [0, cei], [m, Gi], [1, m]]
                        qb = _mk(qt, qoff, [QW, 128], qdims)

                        sL = x_ap(rL)
                        sH = x_ap(rH)
                        d = bfly.tile([128, FD_CAP], HP, tag="d")
                        qd = bfly.tile([128, FD_CAP], HP, tag="qd")
                        dp = s_ap(d)
                        qdp = s_ap(qd)
                        nc.vector.tensor_tensor(dp, sL, sH, O.subtract)
                        nc.vector.tensor_tensor(qdp, dp, qb, O.mult)
                        if swap:
                            nc.vector.tensor_tensor(sH, sH, qdp, O.add)
                            nc.vector.tensor_tensor(sL, sL, qdp, O.subtract)
                        else:
                            d2 = bfly.tile([128, FD_CAP // 2], HP, tag="d2")
                            d2p = _mk(d2[:].tensor, 0, [FD_CAP // 2, 128],
                                      [[cei * Gi * m, nWi], [Gi * m, cei],
                                       [m, Gi], [1, m]])
                            nc.vector.tensor_tensor(d2p, dp, qdp, O.subtract)
                            nc.vector.tensor_tensor(sL, sH, qdp, O.add)
                            nc.vector.tensor_tensor(sH, sH, d2p, O.add)
            if swap and ic in (None, 1):
                bstate["M"] = M ^ m

        # ---- interleaved schedule: q-chunk k+LOOKAHEAD is emitted before
        # the butterflies of chunk k, so ACT computes q ahead of DVE ----
        nchunks = (n_layers + Q_CHUNK - 1) // Q_CHUNK
        LOOKAHEAD = 2
        for k in range(min(LOOKAHEAD, nchunks)):
            emit_q_chunk(k * Q_CHUNK)
        # x_out dram [2, 64, 256, 128] = (jh, b, i, jl); jh=1 needs ir-halves
        # swapped back (global i = ir ^ 128).
        o0 = x_out[0]
        o1 = x_out[1]

        def emit_out_dma(half):
            # half 0: ir<128 rows final -> out[0] i<128, out[1] i>=128
            lo, hi = (0, 128 * 128) if half == 0 else (128 * 128, XF)
            o0s = o0[:, 0:128] if half == 0 else o0[:, 128:256]
            o1s = o1[:, 128:256] if half == 0 else o1[:, 0:128]
            nc.sync.dma_start(o0s.rearrange("b i j -> b (i j)"), x[0:64, lo:hi])
            nc.sync.dma_start(o1s.rearrange("b i j -> b (i j)"), x[64:128, lo:hi])

        last = n_layers - 1
        split_last = n_layers == L  # layer 35 is w=256: split + interleave DMA
        for k in range(nchunks):
            if k + LOOKAHEAD < nchunks:
                emit_q_chunk((k + LOOKAHEAD) * Q_CHUNK)
            for t in range(k * Q_CHUNK, min((k + 1) * Q_CHUNK, n_layers)):
                if t == last and split_last:
                    emit_bfly_layer(t, ic=0)
                    emit_out_dma(0)
                    emit_bfly_layer(t, ic=1)
                    emit_out_dma(1)
                else:
                    emit_bfly_layer(t)
        assert n_layers < L or bstate["M"] == 0, f"final XOR mask {bstate['M']}"
        if not split_last:
            emit_out_dma(0)
            emit_out_dma(1)


def build_nc(n_layers=L):
    nc = bacc.Bacc("TRN2", target_bir_lowering=False, debug=False)
    v_in = nc.declare_dram_parameter("vectors", [B_LOC, N], FP, isOutput=False)
    x_out = nc.declare_dram_parameter("out", [2, B_LOC, N, 128], HP, isOutput=True)
    with tile.TileContext(nc) as tc:
        emit(tc, v_in[:], x_out[:], n_layers=n_layers)
    nc.finalize()
    return nc


_NC_CACHE = {}


def kernel(**inputs) -> np.ndarray:
    vectors = np.asarray(inputs["vectors"], dtype=np.float32)
    assert vectors.shape == (B_FULL, N)
    if "default" not in _NC_CACHE:
        _NC_CACHE["default"] = build_nc()
    nc = _NC_CACHE["default"]
    in_maps = [
        {"vectors": vectors[c * B_LOC : (c + 1) * B_LOC]} for c in range(N_CORES)
    ]
    res = run_bass_kernel_spmd(nc, in_maps, core_ids=list(range(N_CORES)))
    outs = []
    for c in range(N_CORES):
        o = np.asarray(res.results[c]["out"])  # [2, 64, 256, 128] fp16
        outs.append(
            np.transpose(o, (1, 2, 0, 3)).reshape(B_LOC, N, N).astype(np.float32)
        )
    return np.concatenate(outs, axis=0)


if __name__ == "__main__":
    rng = np.random.default_rng(0)
    v = rng.normal(size=(B_FULL, N)).astype(np.float32)
    o = kernel(vectors=v)
    print("kernel output shape:", o.shape, o.dtype)
